# revision 2
# baseline (speedup 1.0000x reference)
"""Trainium2 Bass kernel for nn_NewAttention (sparse gaussian attention).

Reference computation (B=8, L=1024, E=1024, H=8, d=128):
    v    = (values @ Wi.T)                        # [B, L, E] per-position projection
    v    = where(key_mask, 0, v)                  # zero masked key rows
    att  = einsum('hqv,bhvd->bhqd', gauss, v_h)   # per-head gaussian positional conv
    out  = att_merged @ Wo.T                      # [B, L, E]

The gaussian weights w[h,q,v] = pdf(v - ofs_h - q; std=1) form a narrow band
(|diff| <~ 13 before fp32 underflow), so the attention is a per-head 1-D
convolution along the sequence.  Strategy per core (data-parallel over B,
one batch per NeuronCore, no collectives):

  mm1:  vp[v, c] = AT[e, v].T @ WiT[e, c]          (AT = values[b].T, host-prepped)
  shift: SBUF->SBUF DMA vp into vs[u] = vp rows [128u-64, 128u+64) so any
         128-row window needed by the band is one partition-aligned tile.
  conv: att_T[c, q-chunk t] = vs[t].T @ T1[h] + vs[t+1].T @ T2[h]
         where T1/T2 are 128x128 Toeplitz slices of the gaussian band
         (host-precomputed, exact; covers |diff| <= 64 >> fp32 underflow).
  mm2:  out[q, m] = att_T[c, q].T @ WoT[c, m]      (accumulated over head chunks)

All matmuls fp32 (PE does 4-pass fp32 internally -> full fp32 accuracy).
Host pre-transposes values/Wi/Wo so the device never transposes anything.
"""

import math
import sys
import types

import numpy as np

import concourse.bacc as bacc
import concourse.bass as bass
import concourse.mybir as mybir
import concourse.tile as tile
from concourse.bass import ts
from concourse.bass_utils import run_bass_kernel_spmd

B = 8
L = 1024
E = 1024
H = 8
P = 128
NT = L // P  # 8 seq chunks
FP32 = mybir.dt.float32

ATTN_OFFSET = [-3, -2, -1, 0, 0, 1, 2, 3]


def _build_toeplitz():
    """T1[h][j, i] = gauss((j - 64) - i - ofs_h), T2: (j + 64) instead.

    Window W1(t) = vp rows [128t-64, 128t+64), W2(t) = [128t+64, 128t+192).
    att[q0+i, c_h] = sum_j vs[t][j] * T1[h][j, i] + vs[t+1][j] * T2[h][j, i].
    """
    j = np.arange(P, dtype=np.float64)[:, None]
    i = np.arange(P, dtype=np.float64)[None, :]
    t1 = np.zeros((H, P, P), np.float32)
    t2 = np.zeros((H, P, P), np.float32)
    inv_sqrt2pi = 1.0 / math.sqrt(2.0 * math.pi)
    for h in range(H):
        d1 = (j - 64.0) - i - ATTN_OFFSET[h]
        d2 = (j + 64.0) - i - ATTN_OFFSET[h]
        t1[h] = (np.exp(-0.5 * d1 * d1) * inv_sqrt2pi).astype(np.float32)
        t2[h] = (np.exp(-0.5 * d2 * d2) * inv_sqrt2pi).astype(np.float32)
    return t1, t2


_CACHE = {}


def _build_program():
    if "nc" in _CACHE:
        return _CACHE["nc"]

    nc = bacc.Bacc("TRN2", debug=False, num_devices=B)

    at_d = nc.dram_tensor("at", [E, L], FP32, kind="ExternalInput")
    wit_d = nc.dram_tensor("wit", [E, E], FP32, kind="ExternalInput")
    wot_d = nc.dram_tensor("wot", [E, E], FP32, kind="ExternalInput")
    t1_d = nc.dram_tensor("t1", [H, P, P], FP32, kind="ExternalInput")
    t2_d = nc.dram_tensor("t2", [H, P, P], FP32, kind="ExternalInput")
    out_d = nc.dram_tensor("out", [L, E], FP32, kind="ExternalOutput")

    with tile.TileContext(nc) as tc:
        with (
            tc.tile_pool(name="resident", bufs=1) as rpool,
            tc.tile_pool(name="vp_roll", bufs=2) as vppool,
            tc.tile_pool(name="out_roll", bufs=2) as outpool,
            tc.tile_pool(name="big_ps", bufs=3, space="PSUM") as big_psum,
            tc.tile_pool(name="att_ps", bufs=2, space="PSUM") as att_psum,
        ):
            at_sb = rpool.tile([P, NT, L], FP32, name="at_sb")
            wit_sb = rpool.tile([P, NT, E], FP32, name="wit_sb")
            wot_sb = rpool.tile([P, H, E], FP32, name="wot_sb")
            t1_sb = rpool.tile([P, H, P], FP32, name="t1_sb")
            t2_sb = rpool.tile([P, H, P], FP32, name="t2_sb")
            vs = rpool.tile([P, NT + 1, L], FP32, name="vs")
            att_sb = rpool.tile([P, H, L], FP32, name="att_sb")

            # ── input loads ──
            for k in range(NT):
                nc.sync.dma_start(at_sb[:, k, :], at_d[ts(k, P), :])
                nc.sync.dma_start(wit_sb[:, k, :], wit_d[ts(k, P), :])
            for h in range(H):
                nc.sync.dma_start(t1_sb[:, h, :], t1_d[h])
                nc.sync.dma_start(t2_sb[:, h, :], t2_d[h])
                nc.sync.dma_start(wot_sb[:, h, :], wot_d[ts(h, P), :])
            # zero-pad rows beyond the sequence edges
            nc.gpsimd.memset(vs[0:64, 0, :], 0.0)
            nc.gpsimd.memset(vs[64:P, NT, :], 0.0)

            # ── mm1: vp[v-block t] = AT.T @ WiT, accumulate over e-chunks k ──
            for t in range(NT):
                vp_ps = big_psum.tile([P, E], FP32, name="big_ps", tag="big")
                vp_ps0 = vp_ps[:, 0:512]
                vp_ps1 = vp_ps[:, 512:E]
                for k in range(NT):
                    nc.tensor.matmul(
                        vp_ps0[:],
                        at_sb[:, k, ts(t, P)],
                        wit_sb[:, k, 0:512],
                        start=(k == 0),
                        stop=(k == NT - 1),
                    )
                    nc.tensor.matmul(
                        vp_ps1[:],
                        at_sb[:, k, ts(t, P)],
                        wit_sb[:, k, 512:E],
                        start=(k == 0),
                        stop=(k == NT - 1),
                    )
                vp_t = vppool.tile([P, E], FP32, name="vp_t")
                nc.vector.tensor_copy(vp_t[:, 0:512], vp_ps0[:])
                nc.vector.tensor_copy(vp_t[:, 512:E], vp_ps1[:])
                # scatter into 64-shifted windows: vs[u] = vp rows [128u-64, 128u+64)
                nc.sync.dma_start(vs[64:P, t, :], vp_t[0:64, :])
                nc.sync.dma_start(vs[0:64, t + 1, :], vp_t[64:P, :])

            # ── conv: att_T[c_h, q] = vs[t].T @ T1[h] + vs[t+1].T @ T2[h] ──
            for h in range(H):
                for half in range(2):
                    att_ps = att_psum.tile([P, 512], FP32, name="att_ps")
                    for tt in range(4):
                        t = 4 * half + tt
                        nc.tensor.matmul(
                            att_ps[:, ts(tt, P)],
                            vs[:, t, ts(h, P)],
                            t1_sb[:, h, :],
                            start=True,
                            stop=False,
                        )
                        nc.tensor.matmul(
                            att_ps[:, ts(tt, P)],
                            vs[:, t + 1, ts(h, P)],
                            t2_sb[:, h, :],
                            start=False,
                            stop=True,
                        )
                    nc.vector.tensor_copy(att_sb[:, h, ts(half, 512)], att_ps[:])

            # ── mm2: out[q-block t] = att_T.T @ WoT, accumulate over heads ──
            for t in range(NT):
                out_ps = big_psum.tile([P, E], FP32, name="big_ps", tag="big")
                out_ps0 = out_ps[:, 0:512]
                out_ps1 = out_ps[:, 512:E]
                for h in range(H):
                    nc.tensor.matmul(
                        out_ps0[:],
                        att_sb[:, h, ts(t, P)],
                        wot_sb[:, h, 0:512],
                        start=(h == 0),
                        stop=(h == H - 1),
                    )
                    nc.tensor.matmul(
                        out_ps1[:],
                        att_sb[:, h, ts(t, P)],
                        wot_sb[:, h, 512:E],
                        start=(h == 0),
                        stop=(h == H - 1),
                    )
                out_t = outpool.tile([P, E], FP32, name="out_t")
                nc.vector.tensor_copy(out_t[:, 0:512], out_ps0[:])
                nc.vector.tensor_copy(out_t[:, 512:E], out_ps1[:])
                nc.sync.dma_start(out_d[ts(t, P), :], out_t[:])

    nc.compile()
    _CACHE["nc"] = nc
    return nc


def _make_in_maps(values, key_mask, input_weights, output_weight):
    t1, t2 = _build_toeplitz()
    wit = np.ascontiguousarray(input_weights.T).astype(np.float32, copy=False)
    wot = np.ascontiguousarray(output_weight.T).astype(np.float32, copy=False)
    keep = (~np.asarray(key_mask, dtype=bool)).astype(np.float32)
    in_maps = []
    for b in range(B):
        at = np.ascontiguousarray(
            (np.asarray(values[b], np.float32) * keep[b][:, None]).T
        )
        in_maps.append({"at": at, "wit": wit, "wot": wot, "t1": t1, "t2": t2})
    return in_maps


def _run(values, key_mask, input_weights, output_weight, trace=False):
    nc = _build_program()
    in_maps = _make_in_maps(values, key_mask, input_weights, output_weight)
    res = run_bass_kernel_spmd(nc, in_maps, core_ids=list(range(B)), trace=trace)
    out = np.stack([np.asarray(res.results[b]["out"]) for b in range(B)], axis=0)
    return out.astype(np.float32, copy=False), res


def kernel(values, queries, key_mask, input_weights, output_weight):
    out, _ = _run(values, key_mask, input_weights, output_weight, trace=False)
    return out


# revision 3
# speedup vs baseline: 1.1220x; 1.1220x over previous
"""Trainium2 Bass kernel for nn_NewAttention (sparse gaussian attention).

Reference computation (B=8, L=1024, E=1024, H=8, d=128):
    v    = (values @ Wi.T)                        # [B, L, E] per-position projection
    v    = where(key_mask, 0, v)                  # zero masked key rows
    att  = einsum('hqv,bhvd->bhqd', gauss, v_h)   # per-head gaussian positional conv
    out  = att_merged @ Wo.T                      # [B, L, E]

The gaussian weights w[h,q,v] = pdf(v - ofs_h - q; std=1) form a narrow band
(contributions below fp32 significance for |v - ofs_h - q| > ~13), so the
attention is a per-head 1-D convolution along the sequence.  All heads share
the SAME centered gaussian; the per-head integer offset ofs_h only shifts
where the result is read.  Strategy per core (data-parallel over B, one batch
per NeuronCore, no collectives):

  mm1:  vp_T[c, v] = WiT[e, c].T @ AT[e, v]     (AT = values[b].T, host-prepped;
        PE matmul, fp32, accumulate over e-chunks; output [c-block, v] layout)
  conv: U[c, j] = sum_k g(k) * vp_T[c, j + k], j in [-3, 1027), k in [-5, 5]
        -- 11 fused multiply-add ops per head block on the (otherwise idle)
        VectorE, reading a zero-padded copy of vp_T at shifted free offsets.
  mm2:  out[q, m] = U[c, q + ofs_h].T @ WoT[c, m], accumulated over head
        chunks; the per-head shift is just a free-dim offset of the lhsT.

All matmuls fp32 (PE LOW_HIGH 2-pass -> full fp32 accuracy).  PSUM->SBUF
copies run on ScalarE to keep VectorE free for the convolution.  Host
pre-transposes values/Wi/Wo so the device never transposes anything.
"""

import math

import numpy as np

import concourse.bacc as bacc
import concourse.mybir as mybir
import concourse.tile as tile
from concourse.bass import ts
from concourse.bass_utils import run_bass_kernel_spmd

B = 8
L = 1024
E = 1024
H = 8
P = 128
NT = L // P  # 8 chunks of 128
FP32 = mybir.dt.float32
Alu = mybir.AluOpType

ATTN_OFFSET = [-3, -2, -1, 0, 0, 1, 2, 3]
KTAP = 5  # taps k in [-KTAP, KTAP]; dropped tail < 2e-8 relative
VPAD = 16  # zero pad on each side of vp_T's free dim
ULEN = L + 6  # U computed for j in [-3, L+3)

_CACHE = {}


def _build_program():
    if "nc" in _CACHE:
        return _CACHE["nc"]

    nc = bacc.Bacc("TRN2", debug=False, num_devices=B)

    at_d = nc.dram_tensor("at", [E, L], FP32, kind="ExternalInput")
    wit_d = nc.dram_tensor("wit", [E, E], FP32, kind="ExternalInput")
    wot_d = nc.dram_tensor("wot", [E, E], FP32, kind="ExternalInput")
    out_d = nc.dram_tensor("out", [L, E], FP32, kind="ExternalOutput")

    gval = [
        math.exp(-0.5 * k * k) / math.sqrt(2.0 * math.pi)
        for k in range(-KTAP, KTAP + 1)
    ]

    with tile.TileContext(nc) as tc:
        with (
            tc.tile_pool(name="resident", bufs=1) as rpool,
            tc.tile_pool(name="out_roll", bufs=2) as outpool,
            tc.tile_pool(name="big_ps", bufs=3, space="PSUM") as big_psum,
        ):
            at_sb = rpool.tile([P, NT, L], FP32, name="at_sb")
            wit_sb = rpool.tile([P, NT, E], FP32, name="wit_sb")
            wot_sb = rpool.tile([P, H, E], FP32, name="wot_sb")
            vpt = rpool.tile([P, H, L + 2 * VPAD], FP32, name="vpt")
            u = rpool.tile([P, H, ULEN], FP32, name="u")

            # input loads
            for k in range(NT):
                nc.sync.dma_start(at_sb[:, k, :], at_d[ts(k, P), :])
                nc.sync.dma_start(wit_sb[:, k, :], wit_d[ts(k, P), :])
                nc.sync.dma_start(wot_sb[:, k, :], wot_d[ts(k, P), :])
            # zero the vp_T sequence-edge padding (read at shifts up to +-8)
            for cb in range(H):
                nc.gpsimd.memset(vpt[:, cb, 0:VPAD], 0.0)
                nc.gpsimd.memset(vpt[:, cb, VPAD + L : L + 2 * VPAD], 0.0)

            # mm1 + per-block conv taps, pipelined per c-block (= head)
            for cb in range(H):
                vp_ps = big_psum.tile([P, E], FP32, name="big_ps", tag="big")
                for k in range(NT):
                    nc.tensor.matmul(
                        vp_ps[:, 0:512],
                        wit_sb[:, k, ts(cb, P)],
                        at_sb[:, k, 0:512],
                        start=(k == 0),
                        stop=(k == NT - 1),
                    )
                    nc.tensor.matmul(
                        vp_ps[:, 512:E],
                        wit_sb[:, k, ts(cb, P)],
                        at_sb[:, k, 512:E],
                        start=(k == 0),
                        stop=(k == NT - 1),
                    )
                nc.scalar.copy(vpt[:, cb, VPAD : VPAD + 512], vp_ps[:, 0:512])
                nc.scalar.copy(vpt[:, cb, VPAD + 512 : VPAD + L], vp_ps[:, 512:E])

                # conv taps for this head block: U[j] = sum_k g(k) vp[j+k],
                # j in [-3, L+3) stored at u[:, cb, j+3]
                base = VPAD - 3 - KTAP  # vpt col of vp[j + k] at j=-3, k=-KTAP
                nc.vector.tensor_scalar_mul(
                    u[:, cb, :], vpt[:, cb, base : base + ULEN], gval[0]
                )
                for ki in range(1, 2 * KTAP + 1):
                    nc.vector.scalar_tensor_tensor(
                        u[:, cb, :],
                        vpt[:, cb, base + ki : base + ki + ULEN],
                        gval[ki],
                        u[:, cb, :],
                        Alu.mult,
                        Alu.add,
                    )

            # mm2: out[q-block t] = U_shifted.T @ WoT, accumulate over heads
            for t in range(NT):
                out_ps = big_psum.tile([P, E], FP32, name="big_ps", tag="big")
                for h in range(H):
                    off = 3 + ATTN_OFFSET[h] + t * P
                    nc.tensor.matmul(
                        out_ps[:, 0:512],
                        u[:, h, off : off + P],
                        wot_sb[:, h, 0:512],
                        start=(h == 0),
                        stop=(h == H - 1),
                    )
                    nc.tensor.matmul(
                        out_ps[:, 512:E],
                        u[:, h, off : off + P],
                        wot_sb[:, h, 512:E],
                        start=(h == 0),
                        stop=(h == H - 1),
                    )
                out_t = outpool.tile([P, E], FP32, name="out_t")
                nc.scalar.copy(out_t[:, 0:512], out_ps[:, 0:512])
                nc.scalar.copy(out_t[:, 512:E], out_ps[:, 512:E])
                nc.sync.dma_start(out_d[ts(t, P), :], out_t[:])

    nc.compile()
    _CACHE["nc"] = nc
    return nc


def _make_in_maps(values, key_mask, input_weights, output_weight):
    wit = np.ascontiguousarray(np.asarray(input_weights, np.float32).T)
    wot = np.ascontiguousarray(np.asarray(output_weight, np.float32).T)
    keep = (~np.asarray(key_mask, dtype=bool)).astype(np.float32)
    in_maps = []
    for b in range(B):
        at = np.ascontiguousarray(
            (np.asarray(values[b], np.float32) * keep[b][:, None]).T
        )
        in_maps.append({"at": at, "wit": wit, "wot": wot})
    return in_maps


def _run(values, key_mask, input_weights, output_weight, trace=False):
    nc = _build_program()
    in_maps = _make_in_maps(values, key_mask, input_weights, output_weight)
    res = run_bass_kernel_spmd(nc, in_maps, core_ids=list(range(B)), trace=trace)
    out = np.stack([np.asarray(res.results[b]["out"]) for b in range(B)], axis=0)
    return out.astype(np.float32, copy=False), res


def kernel(values, queries, key_mask, input_weights, output_weight):
    out, _ = _run(values, key_mask, input_weights, output_weight, trace=False)
    return out
